# revision 3
# baseline (speedup 1.0000x reference)
"""Trainium2 Bass kernel for the DigitConvolutionalModel problem.

Math: out = relu(conv3x3(x) @ fc1_w.T + fc1_b) @ fc2_w.T + fc2_b
The 3x3 valid conv followed by a dense layer composes into a single
linear map, so conv_w and fc1_w are folded on the host into one
W1eff [128, 784] matrix. The device then runs two matmuls + bias/relu.

Sharding: pure data parallelism - batch split across 8 cores.

Precision: single fp16 products everywhere (x, W1eff, h, W2 all fp16;
PSUM accumulates f32). End-to-end rel max err ~5e-4 vs the 2e-2 gate.
This halves HBM traffic vs an fp16 hi+lo compensated scheme and cuts
fc1 to 7 matmuls per 512-chunk, so the kernel sits right at the
DMA roofline (~13 MB per core at ~360 GB/s).

Layout: x is staged per-core as [NBT, 128, 6, BT] fp16 where element
[i, p, c, j] = x_t[c*128 + p, i*BT + j]; each batch tile is then one
128-descriptor DMA with ~2*6*BT contiguous bytes per partition. The
16 leftover K rows (768:784) ship once as a [16, 8192] tail tensor.

PE p-state: the tensor engine only reaches 2.4 GHz after 3 us of
continuous execution; warmup matmuls on the weight tile keep it busy
(and ramped) while the first x tiles stream in, and small dummy pads
between batch tiles stop it from outrunning the DMA stream (a stall
would reset the ramp to 1.2 GHz).
"""

import numpy as np

import concourse.bacc as bacc
import concourse.mybir as mybir
import concourse.tile as tile
from concourse.bass_utils import run_bass_kernel_spmd

N_CORES = 8
B = 65536
B_LOCAL = B // N_CORES  # 8192
K = 784                 # input features (28*28)
KM = 768                # main K rows (6 chunks of 128)
KT = 16                 # tail rows
M1 = 128                # fc1 out
M2 = 10                 # fc2 out
NKC = 6                 # main K chunks

F32 = mybir.dt.float32
FP16 = mybir.dt.float16

BT = 1024               # batch tile per DMA
NBT = B_LOCAL // BT     # 8
NS = 512                # matmul moving-dim subtile (one PSUM bank)

# last batch tile's x DMA is split so only a small slice arrives last
LAST_SPLIT = 256

# PE pacing knobs (tuned against TimelineSim)
N_WARM = 24             # warmup matmuls before the first real chunk
PAD_PER_BT = 0          # dummy matmuls appended after each bt's chains

_cache = {}


def _build_nc():
    nc = bacc.Bacc("TRN2", target_bir_lowering=False, debug=False,
                   num_devices=N_CORES)

    x_d = nc.dram_tensor("x_p", [NBT, 128, NKC, BT], FP16,
                         kind="ExternalInput")
    xt_d = nc.dram_tensor("x_tail", [KT, B_LOCAL], FP16,
                          kind="ExternalInput")
    # cols 0:768 = six [128,128] W1 chunks; rows 0:16 of 768:896 = tail
    # weight; 896:906 = W2
    w_d = nc.dram_tensor("w_pack", [128, 906], FP16, kind="ExternalInput")
    # col 0 = b1, col 1 rows 0:10 = b2
    b_d = nc.dram_tensor("b_pack", [128, 2], F32, kind="ExternalInput")
    z_d = nc.dram_tensor("z_t", [M2, B_LOCAL], FP16, kind="ExternalOutput")

    with tile.TileContext(nc) as tc:
        with (
            tc.tile_pool(name="static", bufs=1) as sp,
            tc.tile_pool(name="xp", bufs=NBT) as xp,
            tc.tile_pool(name="hp", bufs=4) as hp,
            tc.tile_pool(name="zp", bufs=3) as zp,
            tc.tile_pool(name="pp1", bufs=3, space="PSUM") as pp1,
            tc.tile_pool(name="pp2", bufs=2, space="PSUM") as pp2,
            tc.tile_pool(name="ppd", bufs=1, space="PSUM") as ppd,
        ):
            # small static loads ride the SP HWDGE queue so their descriptor
            # generation overlaps the x-stream generation on GPSIMD SWDGE
            w = sp.tile([128, 906], FP16, tag="w")
            nc.sync.dma_start(w[:], w_d[:])
            b = sp.tile([128, 2], F32, tag="b")
            nc.sync.dma_start(b[:], b_d[:])
            xtail = sp.tile([KT, B_LOCAL], FP16, tag="xtail")
            nc.sync.dma_start(xtail[:], xt_d[:])

            b1 = b[:, 0:1]
            b2 = b[0:M2, 1:2]
            w1cs = [w[:, c * 128:(c + 1) * 128] for c in range(NKC)]
            wtl = w[0:KT, 768:896]
            w2 = w[:, 896:906]

            # x stream: all batch tiles issued up front (they all fit in
            # SBUF); bus serialization + tile pool deps do the pacing.
            xts = []
            for i in range(NBT):
                xt_t = xp.tile([128, NKC, BT], FP16, tag="x")
                if i == NBT - 1 and LAST_SPLIT:
                    bs = BT - LAST_SPLIT
                    nc.gpsimd.dma_start(xt_t[:, :, 0:bs], x_d[i, :, :, 0:bs])
                    nc.gpsimd.dma_start(xt_t[:, :, bs:BT], x_d[i, :, :, bs:BT])
                else:
                    nc.gpsimd.dma_start(xt_t[:], x_d[i])
                xts.append(xt_t)

            # PE warmup: keep the tensor engine busy (and its p-state
            # ramped) on junk matmuls while bt0 streams in.
            psd = ppd.tile([M1, NS], F32, tag="psd")
            for _ in range(N_WARM):
                nc.tensor.matmul(psd[:], w[:, 0:128], w[:, 0:NS],
                                 start=True, stop=True, skip_group_check=True)

            # fc2 of chunk i is deferred until after chunk i+1's fc1 so the
            # PE never waits on ACT's h output.
            pending = []

            def flush_pending():
                for h_t, goff, n in pending:
                    ps2 = pp2.tile([M2, NS], F32, tag="ps2")
                    nc.tensor.matmul(ps2[:, 0:n], w2, h_t[:],
                                     start=True, stop=True)
                    zt = zp.tile([M2, NS], FP16, tag="z")
                    nc.vector.tensor_scalar_add(zt[:, 0:n], ps2[:, 0:n], b2)
                    nc.sync.dma_start(z_d[:, goff:goff + n], zt[:, 0:n])
                pending.clear()

            for i in range(NBT):
                # chunk schedule within the bt; finish with a small chunk on
                # the last bt so the post-last-DMA drain is short
                if i == NBT - 1 and LAST_SPLIT:
                    csched = [NS] * ((BT - LAST_SPLIT) // NS) + [LAST_SPLIT]
                else:
                    csched = [NS] * (BT // NS)
                off = 0
                for n in csched:
                    goff = i * BT + off
                    ps1 = pp1.tile([M1, NS], F32, tag="ps1")
                    for c in range(NKC):
                        nc.tensor.matmul(
                            ps1[:, 0:n], w1cs[c],
                            xts[i][:, c, off:off + n],
                            start=(c == 0), stop=False)
                    nc.tensor.matmul(ps1[:, 0:n], wtl,
                                     xtail[:, goff:goff + n],
                                     start=False, stop=True)
                    h = hp.tile([M1, NS], FP16, tag="h")
                    nc.scalar.activation(
                        h[:, 0:n], ps1[:, 0:n],
                        mybir.ActivationFunctionType.Relu, bias=b1)
                    flush_pending()
                    pending.append((h[:, 0:n], goff, n))
                    off += n
                for _ in range(PAD_PER_BT if i < NBT - 1 else 0):
                    nc.tensor.matmul(psd[:], w[:, 0:128], w[:, 0:NS],
                                     start=True, stop=True,
                                     skip_group_check=True)
            flush_pending()
    nc.compile()
    return nc


def _fold_weights(conv_w, fc1_w):
    """Fold 3x3 valid cross-correlation + fc1 into one [128, 784] matrix."""
    cw = np.asarray(conv_w, np.float64)
    f1 = np.asarray(fc1_w, np.float64).reshape(M1, 26, 26)
    W = np.zeros((M1, 28, 28), np.float64)
    for di in range(3):
        for dj in range(3):
            W[:, di:di + 26, dj:dj + 26] += cw[di, dj] * f1
    return W.reshape(M1, K).astype(np.float32)


def kernel(x, conv_w, fc1_w, fc1_b, fc2_w, fc2_b):
    if "nc" not in _cache:
        _cache["nc"] = _build_nc()
    nc = _cache["nc"]

    w1t = _fold_weights(conv_w, fc1_w).T.astype(np.float16)  # [784, 128]
    w_pack = np.zeros((128, 906), np.float16)
    for c in range(NKC):
        w_pack[:, c * 128:(c + 1) * 128] = w1t[c * 128:(c + 1) * 128, :]
    w_pack[0:KT, 768:896] = w1t[KM:K, :]
    w_pack[:, 896:906] = np.asarray(fc2_w, np.float32).T.astype(np.float16)
    b_pack = np.zeros((128, 2), np.float32)
    b_pack[:, 0] = np.asarray(fc1_b, np.float32)
    b_pack[0:M2, 1] = np.asarray(fc2_b, np.float32)

    x = np.asarray(x, np.float32)
    in_maps = []
    for cid in range(N_CORES):
        xs = x[cid * B_LOCAL:(cid + 1) * B_LOCAL].T.astype(np.float16)
        # [768, 8192] -> [NBT, 128, 6, BT]: [i,p,c,j] = xs[c*128+p, i*BT+j]
        xm = xs[:KM].reshape(NKC, 128, NBT, BT).transpose(2, 1, 0, 3)
        in_maps.append({
            "x_p": np.ascontiguousarray(xm),
            "x_tail": np.ascontiguousarray(xs[KM:K]),
            "w_pack": w_pack, "b_pack": b_pack,
        })
    res = run_bass_kernel_spmd(nc, in_maps, list(range(N_CORES)))
    outs = [res.results[c]["z_t"].T for c in range(N_CORES)]
    return np.ascontiguousarray(
        np.concatenate(outs, axis=0).astype(np.float32))


# revision 6
# speedup vs baseline: 1.4803x; 1.4803x over previous
"""Trainium2 Bass kernel for the DigitConvolutionalModel problem.

Math: out = relu(conv3x3(x) @ fc1_w.T + fc1_b) @ fc2_w.T + fc2_b
The 3x3 valid conv followed by a dense layer composes into a single
linear map, so conv_w and fc1_w are folded on the host into one
W1eff [128, 784] matrix. The device then runs two matmuls + bias/relu.

Sharding: pure data parallelism - batch split across 8 cores.

Precision: single fp16 products everywhere (x, W1eff, h, W2 all fp16;
PSUM accumulates f32). End-to-end rel max err ~5e-4 vs the 2e-2 gate.
This halves HBM traffic vs an fp16 hi+lo compensated scheme and cuts
fc1 to 7 matmuls per 512-chunk, so the kernel sits right at the
DMA roofline (~13 MB per core at ~360 GB/s).

Layout: x is staged per-core as [NBT, 128, 6, BT] fp16 where element
[i, p, c, j] = x_t[c*128 + p, i*BT + j]; each batch tile is then one
128-descriptor DMA with ~2*6*BT contiguous bytes per partition. The
16 leftover K rows (768:784) ship once as a [16, 8192] tail tensor.

PE p-state: the tensor engine only reaches 2.4 GHz after 3 us of
continuous execution; warmup matmuls on the weight tile keep it busy
(and ramped) while the first x tiles stream in, and small dummy pads
between batch tiles stop it from outrunning the DMA stream (a stall
would reset the ramp to 1.2 GHz).
"""

import numpy as np

import concourse.bacc as bacc
import concourse.mybir as mybir
import concourse.tile as tile
from concourse.bass_utils import run_bass_kernel_spmd

N_CORES = 8
B = 65536
B_LOCAL = B // N_CORES  # 8192
K = 784                 # input features (28*28)
KM = 768                # main K rows (6 chunks of 128)
KT = 16                 # tail rows
M1 = 128                # fc1 out
M2 = 10                 # fc2 out
NKC = 6                 # main K chunks

F32 = mybir.dt.float32
FP16 = mybir.dt.float16

BT = 1024               # batch tile per DMA
NBT = B_LOCAL // BT     # 8
NS = 512                # matmul moving-dim subtile (one PSUM bank)

# last batch tile's x DMA is split so only a small slice arrives last
LAST_SPLIT = 256

# PE pacing knobs (tuned against TimelineSim). An instruction's matmul
# cost is locked in when its last dependency is satisfied: if the PE is
# idle (or <3us into a busy run) at that moment it pays the low p-state
# rate forever. Warmups bridge the PE from the weight-load until bt0
# lands; pads (which depend on each bt's x tile, so they cost out at
# full speed) stop the PE from outrunning the DMA stream mid-flight.
N_WARM = 7              # warmup matmuls before the first real chunk
PAD_SCHED = [5, 5, 5, 5, 5, 5, 5, 0]  # pads after each bt's chains

_cache = {}


def _build_nc():
    nc = bacc.Bacc("TRN2", target_bir_lowering=False, debug=False,
                   num_devices=N_CORES)

    x_d = nc.dram_tensor("x_p", [NBT, 128, NKC, BT], FP16,
                         kind="ExternalInput")
    xt_d = nc.dram_tensor("x_tail", [KT, B_LOCAL], FP16,
                          kind="ExternalInput")
    # cols 0:768 = six [128,128] W1 chunks; rows 0:16 of 768:896 = tail
    # weight; 896:906 = W2
    w_d = nc.dram_tensor("w_pack", [128, 906], FP16, kind="ExternalInput")
    # col 0 = b1, col 1 rows 0:10 = b2
    b_d = nc.dram_tensor("b_pack", [128, 2], F32, kind="ExternalInput")
    z_d = nc.dram_tensor("z_t", [M2, B_LOCAL], FP16, kind="ExternalOutput")

    with tile.TileContext(nc) as tc:
        with (
            tc.tile_pool(name="static", bufs=1) as sp,
            tc.tile_pool(name="xp", bufs=NBT) as xp,
            tc.tile_pool(name="hp", bufs=4) as hp,
            # z tiles are tiny; never recycle them mid-run (their DMAs sit
            # behind the whole x stream in the shared-bus FIFO)
            tc.tile_pool(name="zp", bufs=18) as zp,
            tc.tile_pool(name="pp1", bufs=3, space="PSUM") as pp1,
            tc.tile_pool(name="pp2", bufs=2, space="PSUM") as pp2,
            tc.tile_pool(name="ppd", bufs=1, space="PSUM") as ppd,
        ):
            # small static loads ride the SP HWDGE queue so their descriptor
            # generation overlaps the x-stream generation on GPSIMD SWDGE
            w = sp.tile([128, 906], FP16, tag="w")
            nc.sync.dma_start(w[:], w_d[:])
            b = sp.tile([128, 2], F32, tag="b")
            nc.sync.dma_start(b[:], b_d[:])
            xtail = sp.tile([KT, B_LOCAL], FP16, tag="xtail")
            nc.sync.dma_start(xtail[:], xt_d[:])

            b1 = b[:, 0:1]
            b2 = b[0:M2, 1:2]
            w1cs = [w[:, c * 128:(c + 1) * 128] for c in range(NKC)]
            wtl = w[0:KT, 768:896]
            w2 = w[:, 896:906]

            # x stream: all batch tiles issued up front (they all fit in
            # SBUF); bus serialization + tile pool deps do the pacing.
            xts = []
            for i in range(NBT):
                xt_t = xp.tile([128, NKC, BT], FP16, tag="x")
                if i == NBT - 1 and LAST_SPLIT:
                    bs = BT - LAST_SPLIT
                    nc.gpsimd.dma_start(xt_t[:, :, 0:bs], x_d[i, :, :, 0:bs])
                    nc.gpsimd.dma_start(xt_t[:, :, bs:BT], x_d[i, :, :, bs:BT])
                else:
                    nc.gpsimd.dma_start(xt_t[:], x_d[i])
                xts.append(xt_t)

            # PE warmup: keep the tensor engine busy (and its p-state
            # ramped) on junk matmuls while bt0 streams in.
            psd = ppd.tile([M1, NS], F32, tag="psd")
            for _ in range(N_WARM):
                nc.tensor.matmul(psd[:], w[:, 0:128], w[:, 0:NS],
                                 start=True, stop=True, skip_group_check=True)

            # fc2 of chunk i is deferred until after chunk i+1's fc1 so the
            # PE never waits on ACT's h output.
            pending = []

            def flush_pending():
                for h_t, goff, n in pending:
                    ps2 = pp2.tile([M2, NS], F32, tag="ps2")
                    nc.tensor.matmul(ps2[:, 0:n], w2, h_t[:],
                                     start=True, stop=True)
                    zt = zp.tile([M2, NS], FP16, tag="z")
                    nc.vector.tensor_scalar_add(zt[:, 0:n], ps2[:, 0:n], b2)
                    nc.sync.dma_start(z_d[:, goff:goff + n], zt[:, 0:n])
                pending.clear()

            for i in range(NBT):
                # chunk schedule within the bt; finish with a small chunk on
                # the last bt so the post-last-DMA drain is short
                if i == NBT - 1 and LAST_SPLIT:
                    csched = [NS] * ((BT - LAST_SPLIT) // NS) + [LAST_SPLIT]
                else:
                    csched = [NS] * (BT // NS)
                off = 0
                for n in csched:
                    goff = i * BT + off
                    ps1 = pp1.tile([M1, NS], F32, tag="ps1")
                    for c in range(NKC):
                        nc.tensor.matmul(
                            ps1[:, 0:n], w1cs[c],
                            xts[i][:, c, off:off + n],
                            start=(c == 0), stop=False)
                    nc.tensor.matmul(ps1[:, 0:n], wtl,
                                     xtail[:, goff:goff + n],
                                     start=False, stop=True)
                    h = hp.tile([M1, NS], FP16, tag="h")
                    nc.scalar.activation(
                        h[:, 0:n], ps1[:, 0:n],
                        mybir.ActivationFunctionType.Relu, bias=b1)
                    flush_pending()
                    pending.append((h[:, 0:n], goff, n))
                    off += n
                for _ in range(PAD_SCHED[i]):
                    nc.tensor.matmul(psd[:], w[:, 0:128],
                                     xts[i][:, 0, 0:NS],
                                     start=True, stop=True,
                                     skip_group_check=True)
            flush_pending()
    nc.compile()
    return nc


def _fold_weights(conv_w, fc1_w):
    """Fold 3x3 valid cross-correlation + fc1 into one [128, 784] matrix."""
    cw = np.asarray(conv_w, np.float64)
    f1 = np.asarray(fc1_w, np.float64).reshape(M1, 26, 26)
    W = np.zeros((M1, 28, 28), np.float64)
    for di in range(3):
        for dj in range(3):
            W[:, di:di + 26, dj:dj + 26] += cw[di, dj] * f1
    return W.reshape(M1, K).astype(np.float32)


def kernel(x, conv_w, fc1_w, fc1_b, fc2_w, fc2_b):
    if "nc" not in _cache:
        _cache["nc"] = _build_nc()
    nc = _cache["nc"]

    w1t = _fold_weights(conv_w, fc1_w).T.astype(np.float16)  # [784, 128]
    w_pack = np.zeros((128, 906), np.float16)
    for c in range(NKC):
        w_pack[:, c * 128:(c + 1) * 128] = w1t[c * 128:(c + 1) * 128, :]
    w_pack[0:KT, 768:896] = w1t[KM:K, :]
    w_pack[:, 896:906] = np.asarray(fc2_w, np.float32).T.astype(np.float16)
    b_pack = np.zeros((128, 2), np.float32)
    b_pack[:, 0] = np.asarray(fc1_b, np.float32)
    b_pack[0:M2, 1] = np.asarray(fc2_b, np.float32)

    x = np.asarray(x, np.float32)
    in_maps = []
    for cid in range(N_CORES):
        xs = x[cid * B_LOCAL:(cid + 1) * B_LOCAL].T.astype(np.float16)
        # [768, 8192] -> [NBT, 128, 6, BT]: [i,p,c,j] = xs[c*128+p, i*BT+j]
        xm = xs[:KM].reshape(NKC, 128, NBT, BT).transpose(2, 1, 0, 3)
        in_maps.append({
            "x_p": np.ascontiguousarray(xm),
            "x_tail": np.ascontiguousarray(xs[KM:K]),
            "w_pack": w_pack, "b_pack": b_pack,
        })
    res = run_bass_kernel_spmd(nc, in_maps, list(range(N_CORES)))
    outs = [res.results[c]["z_t"].T for c in range(N_CORES)]
    return np.ascontiguousarray(
        np.concatenate(outs, axis=0).astype(np.float32))


# revision 10
# speedup vs baseline: 1.5598x; 1.0537x over previous
"""Trainium2 Bass kernel for the DigitConvolutionalModel problem.

Math: out = relu(conv3x3(x) @ fc1_w.T + fc1_b) @ fc2_w.T + fc2_b
The 3x3 valid conv followed by a dense layer composes into a single
linear map, so conv_w and fc1_w are folded on the host into one
W1eff [128, 784] matrix. The device then runs two matmuls + bias/relu.

Sharding: pure data parallelism - batch split across 8 cores.

Precision: single fp16 products everywhere (x, W1eff, h, W2 all fp16;
PSUM accumulates f32). End-to-end rel max err ~5e-4 vs the 2e-2 gate.
This halves HBM traffic vs an fp16 hi+lo compensated scheme and cuts
fc1 to 7 matmuls per 512-chunk, so the kernel sits right at the
DMA roofline (~13 MB per core at ~360 GB/s).

Layout: x is staged per-core as [NBT, 128, 6, BT] fp16 where element
[i, p, c, j] = x_t[c*128 + p, i*BT + j]; each batch tile is then one
128-descriptor DMA with ~2*6*BT contiguous bytes per partition. The
16 leftover K rows (768:784) ship once as a [16, 8192] tail tensor.

PE p-state: the tensor engine only reaches 2.4 GHz after 3 us of
continuous execution; warmup matmuls on the weight tile keep it busy
(and ramped) while the first x tiles stream in, and small dummy pads
between batch tiles stop it from outrunning the DMA stream (a stall
would reset the ramp to 1.2 GHz).
"""

import numpy as np

import concourse.bacc as bacc
import concourse.mybir as mybir
import concourse.tile as tile
from concourse.bass_utils import run_bass_kernel_spmd

N_CORES = 8
B = 65536
B_LOCAL = B // N_CORES  # 8192
K = 784                 # input features (28*28)
KM = 768                # main K rows (6 chunks of 128)
KT = 16                 # tail rows
M1 = 128                # fc1 out
M2 = 10                 # fc2 out
NKC = 6                 # main K chunks

F32 = mybir.dt.float32
FP16 = mybir.dt.float16

BT = 1024               # batch tile per DMA
NBT = B_LOCAL // BT     # 8
NS = 512                # matmul moving-dim subtile (one PSUM bank)

# last batch tile's x DMA is split so only a small slice arrives last
LAST_SPLIT = 256

# PE pacing knobs (tuned against TimelineSim). An instruction's matmul
# cost is locked in when its last dependency is satisfied: if the PE is
# idle (or <3us into a busy run) at that moment it pays the low p-state
# rate forever. Warmups bridge the PE from the weight-load until bt0
# lands; pads (which depend on each bt's x tile, so they cost out at
# full speed) stop the PE from outrunning the DMA stream mid-flight.
N_WARM = 10             # warmup matmuls before the first real chunk
PAD_SCHED = [3, 3, 3, 3, 3, 3, 3, 0]  # pads after each bt's chains

_cache = {}


def _build_nc():
    nc = bacc.Bacc("TRN2", target_bir_lowering=False, debug=False,
                   num_devices=N_CORES)

    x_d = nc.dram_tensor("x_p", [NBT, 128, NKC, BT], FP16,
                         kind="ExternalInput")
    xt_d = nc.dram_tensor("x_tail", [KT, B_LOCAL], FP16,
                          kind="ExternalInput")
    # cols 0:768 = six [128,128] W1 chunks; rows 0:16 of 768:896 = tail
    # weight; 896:906 = W2
    w_d = nc.dram_tensor("w_pack", [128, 906], FP16, kind="ExternalInput")
    # col 0 = b1, col 1 rows 0:10 = b2
    b_d = nc.dram_tensor("b_pack", [128, 2], F32, kind="ExternalInput")
    z_d = nc.dram_tensor("z_t", [M2, B_LOCAL], FP16, kind="ExternalOutput")

    with tile.TileContext(nc) as tc:
        with (
            tc.tile_pool(name="static", bufs=1) as sp,
            tc.tile_pool(name="xp", bufs=NBT) as xp,
            tc.tile_pool(name="hp", bufs=4) as hp,
            # z tiles are tiny; never recycle them mid-run (their DMAs sit
            # behind the whole x stream in the shared-bus FIFO)
            tc.tile_pool(name="zp", bufs=18) as zp,
            tc.tile_pool(name="pp1", bufs=3, space="PSUM") as pp1,
            tc.tile_pool(name="pp2", bufs=2, space="PSUM") as pp2,
            tc.tile_pool(name="ppd", bufs=1, space="PSUM") as ppd,
        ):
            # warmup operand needs no DMA: memset lets the PE start matmuls
            # almost immediately, so its 3us p-state ramp completes before
            # any real matmul's cost is locked in
            wu = sp.tile([128, NS], FP16, tag="wu")
            nc.vector.memset(wu[:], 1.0)

            # small static loads ride the SP HWDGE queue so their descriptor
            # generation overlaps the x-stream generation on GPSIMD SWDGE
            w = sp.tile([128, 906], FP16, tag="w")
            nc.sync.dma_start(w[:], w_d[:])
            b = sp.tile([128, 2], F32, tag="b")
            nc.sync.dma_start(b[:], b_d[:])
            xtail = sp.tile([KT, B_LOCAL], FP16, tag="xtail")
            nc.sync.dma_start(xtail[:], xt_d[:])

            b1 = b[:, 0:1]
            b2 = b[0:M2, 1:2]
            w1cs = [w[:, c * 128:(c + 1) * 128] for c in range(NKC)]
            wtl = w[0:KT, 768:896]
            w2 = w[:, 896:906]

            # x stream: all batch tiles issued up front (they all fit in
            # SBUF); bus serialization + tile pool deps do the pacing.
            xts = []
            for i in range(NBT):
                xt_t = xp.tile([128, NKC, BT], FP16, tag="x")
                if i == NBT - 1 and LAST_SPLIT:
                    bs = BT - LAST_SPLIT
                    nc.gpsimd.dma_start(xt_t[:, :, 0:bs], x_d[i, :, :, 0:bs])
                    nc.gpsimd.dma_start(xt_t[:, :, bs:BT], x_d[i, :, :, bs:BT])
                else:
                    nc.gpsimd.dma_start(xt_t[:], x_d[i])
                xts.append(xt_t)

            # PE warmup: keep the tensor engine busy (and its p-state
            # ramped) on junk matmuls while bt0 streams in.
            psd = ppd.tile([M1, NS], F32, tag="psd")
            for _ in range(N_WARM):
                nc.tensor.matmul(psd[:], wu[:, 0:128], wu[:],
                                 start=True, stop=True, skip_group_check=True)

            # fc2 of chunk i is deferred until after chunk i+1's fc1 so the
            # PE never waits on ACT's h output. z is written per-bt (one
            # DMA each) except the last bt, which flushes per-chunk so the
            # final piece leaves as soon as it exists.
            pending = []

            def flush_pending():
                for h_t, zt_t, lo, n, dma in pending:
                    ps2 = pp2.tile([M2, NS], F32, tag="ps2")
                    nc.tensor.matmul(ps2[:, 0:n], w2, h_t,
                                     start=True, stop=True)
                    nc.vector.tensor_scalar_add(
                        zt_t[:, lo:lo + n], ps2[:, 0:n], b2)
                    if dma is not None:
                        nc.sync.dma_start(*dma)
                pending.clear()

            for i in range(NBT):
                last = i == NBT - 1
                # chunk schedule within the bt; finish with a small chunk on
                # the last bt so the post-last-DMA drain is short
                if last and LAST_SPLIT:
                    csched = [NS] * ((BT - LAST_SPLIT) // NS) + [LAST_SPLIT]
                else:
                    csched = [NS] * (BT // NS)
                if not last:
                    zt = zp.tile([M2, BT], FP16, tag="z")
                off = 0
                for ci, n in enumerate(csched):
                    goff = i * BT + off
                    ps1 = pp1.tile([M1, NS], F32, tag="ps1")
                    for c in range(NKC):
                        nc.tensor.matmul(
                            ps1[:, 0:n], w1cs[c],
                            xts[i][:, c, off:off + n],
                            start=(c == 0), stop=False)
                    nc.tensor.matmul(ps1[:, 0:n], wtl,
                                     xtail[:, goff:goff + n],
                                     start=False, stop=True)
                    h = hp.tile([M1, NS], FP16, tag="h")
                    nc.scalar.activation(
                        h[:, 0:n], ps1[:, 0:n],
                        mybir.ActivationFunctionType.Relu, bias=b1)
                    flush_pending()
                    if last:
                        ztc = zp.tile([M2, NS], FP16, tag="zc")
                        pending.append((h[:, 0:n], ztc, 0, n,
                                        (z_d[:, goff:goff + n], ztc[:, 0:n])))
                    else:
                        dma = ((z_d[:, i * BT:(i + 1) * BT], zt[:])
                               if ci == len(csched) - 1 else None)
                        pending.append((h[:, 0:n], zt, off, n, dma))
                    off += n
                for _ in range(PAD_SCHED[i]):
                    nc.tensor.matmul(psd[:], w[:, 0:128],
                                     xts[i][:, 0, 0:NS],
                                     start=True, stop=True,
                                     skip_group_check=True)
            flush_pending()
    nc.compile()
    return nc


def _fold_weights(conv_w, fc1_w):
    """Fold 3x3 valid cross-correlation + fc1 into one [128, 784] matrix."""
    cw = np.asarray(conv_w, np.float64)
    f1 = np.asarray(fc1_w, np.float64).reshape(M1, 26, 26)
    W = np.zeros((M1, 28, 28), np.float64)
    for di in range(3):
        for dj in range(3):
            W[:, di:di + 26, dj:dj + 26] += cw[di, dj] * f1
    return W.reshape(M1, K).astype(np.float32)


def kernel(x, conv_w, fc1_w, fc1_b, fc2_w, fc2_b):
    if "nc" not in _cache:
        _cache["nc"] = _build_nc()
    nc = _cache["nc"]

    w1t = _fold_weights(conv_w, fc1_w).T.astype(np.float16)  # [784, 128]
    w_pack = np.zeros((128, 906), np.float16)
    for c in range(NKC):
        w_pack[:, c * 128:(c + 1) * 128] = w1t[c * 128:(c + 1) * 128, :]
    w_pack[0:KT, 768:896] = w1t[KM:K, :]
    w_pack[:, 896:906] = np.asarray(fc2_w, np.float32).T.astype(np.float16)
    b_pack = np.zeros((128, 2), np.float32)
    b_pack[:, 0] = np.asarray(fc1_b, np.float32)
    b_pack[0:M2, 1] = np.asarray(fc2_b, np.float32)

    x = np.asarray(x, np.float32)
    in_maps = []
    for cid in range(N_CORES):
        xs = x[cid * B_LOCAL:(cid + 1) * B_LOCAL].T.astype(np.float16)
        # [768, 8192] -> [NBT, 128, 6, BT]: [i,p,c,j] = xs[c*128+p, i*BT+j]
        xm = xs[:KM].reshape(NKC, 128, NBT, BT).transpose(2, 1, 0, 3)
        in_maps.append({
            "x_p": np.ascontiguousarray(xm),
            "x_tail": np.ascontiguousarray(xs[KM:K]),
            "w_pack": w_pack, "b_pack": b_pack,
        })
    res = run_bass_kernel_spmd(nc, in_maps, list(range(N_CORES)))
    outs = [res.results[c]["z_t"].T for c in range(N_CORES)]
    return np.ascontiguousarray(
        np.concatenate(outs, axis=0).astype(np.float32))


# revision 14
# speedup vs baseline: 1.5718x; 1.0077x over previous
"""Trainium2 Bass kernel for the DigitConvolutionalModel problem.

Math: out = relu(conv3x3(x) @ fc1_w.T + fc1_b) @ fc2_w.T + fc2_b
The 3x3 valid conv followed by a dense layer composes into a single
linear map, so conv_w and fc1_w are folded on the host into one
W1eff [128, 784] matrix. The device then runs two matmuls + bias/relu.

Sharding: pure data parallelism - batch split across 8 cores.

Precision: single fp16 products everywhere (x, W1eff, h, W2 all fp16;
PSUM accumulates f32). End-to-end rel max err ~5e-4 vs the 2e-2 gate.
This halves HBM traffic vs an fp16 hi+lo compensated scheme and cuts
fc1 to 7 matmuls per 512-chunk, so the kernel sits right at the
DMA roofline (~13 MB per core at ~360 GB/s).

Layout: x is staged per-core as [NBT, 128, 6, BT] fp16 where element
[i, p, c, j] = x_t[c*128 + p, i*BT + j]; each batch tile is then one
128-descriptor DMA with ~2*6*BT contiguous bytes per partition. The
16 leftover K rows (768:784) ship once as a [16, 8192] tail tensor.

PE p-state: the tensor engine only reaches 2.4 GHz after 3 us of
continuous execution; warmup matmuls on the weight tile keep it busy
(and ramped) while the first x tiles stream in, and small dummy pads
between batch tiles stop it from outrunning the DMA stream (a stall
would reset the ramp to 1.2 GHz).
"""

import numpy as np

import concourse.bacc as bacc
import concourse.mybir as mybir
import concourse.tile as tile
from concourse.bass_utils import run_bass_kernel_spmd

N_CORES = 8
B = 65536
B_LOCAL = B // N_CORES  # 8192
K = 784                 # input features (28*28)
KM = 768                # main K rows (6 chunks of 128)
KT = 16                 # tail rows
M1 = 128                # fc1 out
M2 = 10                 # fc2 out
NKC = 6                 # main K chunks

F32 = mybir.dt.float32
FP16 = mybir.dt.float16

BT = 1024               # batch tile per DMA
NBT = B_LOCAL // BT     # 8
NS = 512                # matmul moving-dim subtile (one PSUM bank)

# last batch tile's x DMA is split so only a small slice arrives last
LAST_SPLIT = 256

# PE pacing knobs (tuned against TimelineSim). An instruction's matmul
# cost is locked in when its last dependency is satisfied: if the PE is
# idle (or <3us into a busy run) at that moment it pays the low p-state
# rate forever. Warmups bridge the PE from the weight-load until bt0
# lands; pads (which depend on each bt's x tile, so they cost out at
# full speed) stop the PE from outrunning the DMA stream mid-flight.
N_WARM = 10             # warmup matmuls before the first real chunk
PAD_SCHED = [3, 3, 3, 3, 3, 3, 3, 0]  # pads after each bt's chains

_cache = {}


def _build_nc():
    nc = bacc.Bacc("TRN2", target_bir_lowering=False, debug=False,
                   num_devices=N_CORES)

    x_d = nc.dram_tensor("x_p", [NBT, 128, NKC, BT], FP16,
                         kind="ExternalInput")
    xt_d = nc.dram_tensor("x_tail", [KT, B_LOCAL], FP16,
                          kind="ExternalInput")
    # cols 0:768 = six [128,128] W1 chunks; rows 0:16 of 768:896 = tail
    # weight; 896:906 = W2
    w_d = nc.dram_tensor("w_pack", [128, 906], FP16, kind="ExternalInput")
    # col 0 = b1, col 1 rows 0:10 = b2
    b_d = nc.dram_tensor("b_pack", [128, 2], F32, kind="ExternalInput")
    z_d = nc.dram_tensor("z_t", [M2, B_LOCAL], FP16, kind="ExternalOutput")

    with tile.TileContext(nc) as tc:
        with (
            tc.tile_pool(name="static", bufs=1) as sp,
            tc.tile_pool(name="xp", bufs=NBT) as xp,
            tc.tile_pool(name="hp", bufs=4) as hp,
            # z accumulates in two one-shot tiles (no recycling: their DMAs
            # sit behind the whole x stream in the shared-bus FIFO)
            tc.tile_pool(name="zp", bufs=1) as zp,
            tc.tile_pool(name="pp1", bufs=3, space="PSUM") as pp1,
            tc.tile_pool(name="pp2", bufs=2, space="PSUM") as pp2,
            tc.tile_pool(name="ppd", bufs=1, space="PSUM") as ppd,
        ):
            # warmup operand needs no DMA: memset lets the PE start matmuls
            # almost immediately, so its 3us p-state ramp completes before
            # any real matmul's cost is locked in
            wu = sp.tile([128, NS], FP16, tag="wu")
            nc.vector.memset(wu[:], 1.0)

            # small static loads ride the SP HWDGE queue so their descriptor
            # generation overlaps the x-stream generation on GPSIMD SWDGE
            w = sp.tile([128, 906], FP16, tag="w")
            nc.sync.dma_start(w[:], w_d[:])
            b = sp.tile([128, 2], F32, tag="b")
            nc.sync.dma_start(b[:], b_d[:])
            xtail = sp.tile([KT, B_LOCAL], FP16, tag="xtail")
            nc.sync.dma_start(xtail[:], xt_d[:])

            b1 = b[:, 0:1]
            b2 = b[0:M2, 1:2]
            w1cs = [w[:, c * 128:(c + 1) * 128] for c in range(NKC)]
            wtl = w[0:KT, 768:896]
            w2 = w[:, 896:906]

            # x stream: all batch tiles issued up front (they all fit in
            # SBUF); bus serialization + tile pool deps do the pacing.
            xts = []
            for i in range(NBT):
                xt_t = xp.tile([128, NKC, BT], FP16, tag="x")
                if i == NBT - 1 and LAST_SPLIT:
                    bs = BT - LAST_SPLIT
                    nc.gpsimd.dma_start(xt_t[:, :, 0:bs], x_d[i, :, :, 0:bs])
                    nc.gpsimd.dma_start(xt_t[:, :, bs:BT], x_d[i, :, :, bs:BT])
                else:
                    nc.gpsimd.dma_start(xt_t[:], x_d[i])
                xts.append(xt_t)

            # PE warmup: keep the tensor engine busy (and its p-state
            # ramped) on junk matmuls while bt0 streams in.
            psd = ppd.tile([M1, NS], F32, tag="psd")
            for _ in range(N_WARM):
                nc.tensor.matmul(psd[:], wu[:, 0:128], wu[:],
                                 start=True, stop=True, skip_group_check=True)

            # fc2 of chunk i is deferred until after chunk i+1's fc1 so the
            # PE never waits on ACT's h output. z accumulates in two SBUF
            # tiles: bts 0-6 go out in one DMA right behind the last x
            # transfer (so no z slice ever delays the x stream on the shared
            # bus), and bt7's goes out the moment its last chunk is ready.
            zA = zp.tile([M2, (NBT - 1) * BT], FP16, tag="zA")
            zB = zp.tile([M2, BT], FP16, tag="zB")
            pending = []

            def flush_pending():
                for h_t, zt_t, lo, n, dma in pending:
                    ps2 = pp2.tile([M2, NS], F32, tag="ps2")
                    nc.tensor.matmul(ps2[:, 0:n], w2, h_t,
                                     start=True, stop=True)
                    nc.vector.tensor_scalar_add(
                        zt_t[:, lo:lo + n], ps2[:, 0:n], b2)
                    if dma is not None:
                        nc.sync.dma_start(*dma)
                pending.clear()

            for i in range(NBT):
                last = i == NBT - 1
                # chunk schedule within the bt; finish with a small chunk on
                # the last bt so the post-last-DMA drain is short
                if last and LAST_SPLIT:
                    csched = [NS] * ((BT - LAST_SPLIT) // NS) + [LAST_SPLIT]
                else:
                    csched = [NS] * (BT // NS)
                off = 0
                for ci, n in enumerate(csched):
                    goff = i * BT + off
                    ps1 = pp1.tile([M1, NS], F32, tag="ps1")
                    for c in range(NKC):
                        nc.tensor.matmul(
                            ps1[:, 0:n], w1cs[c],
                            xts[i][:, c, off:off + n],
                            start=(c == 0), stop=False)
                    nc.tensor.matmul(ps1[:, 0:n], wtl,
                                     xtail[:, goff:goff + n],
                                     start=False, stop=True)
                    h = hp.tile([M1, NS], FP16, tag="h")
                    nc.scalar.activation(
                        h[:, 0:n], ps1[:, 0:n],
                        mybir.ActivationFunctionType.Relu, bias=b1)
                    flush_pending()
                    final_chunk = ci == len(csched) - 1
                    if last:
                        dma = ((z_d[:, i * BT:], zB[:])
                               if final_chunk else None)
                        pending.append((h[:, 0:n], zB, off, n, dma))
                    else:
                        dma = ((z_d[:, 0:(NBT - 1) * BT], zA[:])
                               if i == NBT - 2 and final_chunk else None)
                        pending.append((h[:, 0:n], zA, i * BT + off, n, dma))
                    off += n
                for _ in range(PAD_SCHED[i]):
                    nc.tensor.matmul(psd[:], w[:, 0:128],
                                     xts[i][:, 0, 0:NS],
                                     start=True, stop=True,
                                     skip_group_check=True)
            flush_pending()
    nc.compile()
    return nc


def _fold_weights(conv_w, fc1_w):
    """Fold 3x3 valid cross-correlation + fc1 into one [128, 784] matrix."""
    cw = np.asarray(conv_w, np.float64)
    f1 = np.asarray(fc1_w, np.float64).reshape(M1, 26, 26)
    W = np.zeros((M1, 28, 28), np.float64)
    for di in range(3):
        for dj in range(3):
            W[:, di:di + 26, dj:dj + 26] += cw[di, dj] * f1
    return W.reshape(M1, K).astype(np.float32)


def kernel(x, conv_w, fc1_w, fc1_b, fc2_w, fc2_b):
    if "nc" not in _cache:
        _cache["nc"] = _build_nc()
    nc = _cache["nc"]

    w1t = _fold_weights(conv_w, fc1_w).T.astype(np.float16)  # [784, 128]
    w_pack = np.zeros((128, 906), np.float16)
    for c in range(NKC):
        w_pack[:, c * 128:(c + 1) * 128] = w1t[c * 128:(c + 1) * 128, :]
    w_pack[0:KT, 768:896] = w1t[KM:K, :]
    w_pack[:, 896:906] = np.asarray(fc2_w, np.float32).T.astype(np.float16)
    b_pack = np.zeros((128, 2), np.float32)
    b_pack[:, 0] = np.asarray(fc1_b, np.float32)
    b_pack[0:M2, 1] = np.asarray(fc2_b, np.float32)

    x = np.asarray(x, np.float32)
    in_maps = []
    for cid in range(N_CORES):
        xs = x[cid * B_LOCAL:(cid + 1) * B_LOCAL].T.astype(np.float16)
        # [768, 8192] -> [NBT, 128, 6, BT]: [i,p,c,j] = xs[c*128+p, i*BT+j]
        xm = xs[:KM].reshape(NKC, 128, NBT, BT).transpose(2, 1, 0, 3)
        in_maps.append({
            "x_p": np.ascontiguousarray(xm),
            "x_tail": np.ascontiguousarray(xs[KM:K]),
            "w_pack": w_pack, "b_pack": b_pack,
        })
    res = run_bass_kernel_spmd(nc, in_maps, list(range(N_CORES)))
    outs = [res.results[c]["z_t"].T for c in range(N_CORES)]
    return np.ascontiguousarray(
        np.concatenate(outs, axis=0).astype(np.float32))


# revision 15
# speedup vs baseline: 1.5913x; 1.0124x over previous
"""Trainium2 Bass kernel for the DigitConvolutionalModel problem.

Math: out = relu(conv3x3(x) @ fc1_w.T + fc1_b) @ fc2_w.T + fc2_b
The 3x3 valid conv followed by a dense layer composes into a single
linear map, so conv_w and fc1_w are folded on the host into one
W1eff [128, 784] matrix. The device then runs two matmuls + bias/relu.

Sharding: pure data parallelism - batch split across 8 cores.

Precision: single fp16 products everywhere (x, W1eff, h, W2 all fp16;
PSUM accumulates f32). End-to-end rel max err ~5e-4 vs the 2e-2 gate.
This halves HBM traffic vs an fp16 hi+lo compensated scheme and cuts
fc1 to 7 matmuls per 512-chunk, so the kernel sits right at the
DMA roofline (~13 MB per core at ~360 GB/s).

Layout: x is staged per-core as [NBT, 128, 6, BT] fp16 where element
[i, p, c, j] = x_t[c*128 + p, i*BT + j]; each batch tile is then one
128-descriptor DMA with ~2*6*BT contiguous bytes per partition. The
16 leftover K rows (768:784) ship once as a [16, 8192] tail tensor.

PE p-state: the tensor engine only reaches 2.4 GHz after 3 us of
continuous execution; warmup matmuls on the weight tile keep it busy
(and ramped) while the first x tiles stream in, and small dummy pads
between batch tiles stop it from outrunning the DMA stream (a stall
would reset the ramp to 1.2 GHz).
"""

import numpy as np

import concourse.bacc as bacc
import concourse.mybir as mybir
import concourse.tile as tile
from concourse.bass_utils import run_bass_kernel_spmd

N_CORES = 8
B = 65536
B_LOCAL = B // N_CORES  # 8192
K = 784                 # input features (28*28)
KM = 768                # main K rows (6 chunks of 128)
KT = 16                 # tail rows
M1 = 128                # fc1 out
M2 = 10                 # fc2 out
NKC = 6                 # main K chunks

F32 = mybir.dt.float32
FP16 = mybir.dt.float16

BT = 1024               # batch tile per DMA
NBT = B_LOCAL // BT     # 8
NS = 512                # matmul moving-dim subtile (one PSUM bank)

# last batch tile's x DMA is split so only a small slice arrives last
LAST_SPLIT = 256

# PE pacing knobs (tuned against TimelineSim). An instruction's matmul
# cost is locked in when its last dependency is satisfied: if the PE is
# idle (or <3us into a busy run) at that moment it pays the low p-state
# rate forever. Warmups bridge the PE from the weight-load until bt0
# lands; pads (which depend on each bt's x tile, so they cost out at
# full speed) stop the PE from outrunning the DMA stream mid-flight.
N_WARM = 10             # warmup matmuls before the first real chunk
PAD_SCHED = [0, 0, 0, 0, 0, 0, 0, 0]  # pads after each bt's chains

_cache = {}


def _build_nc():
    nc = bacc.Bacc("TRN2", target_bir_lowering=False, debug=False,
                   num_devices=N_CORES)

    x_d = nc.dram_tensor("x_p", [NBT, 128, NKC, BT], FP16,
                         kind="ExternalInput")
    xt_d = nc.dram_tensor("x_tail", [KT, B_LOCAL], FP16,
                          kind="ExternalInput")
    # cols 0:768 = six [128,128] W1 chunks; rows 0:16 of 768:896 = tail
    # weight; 896:906 = W2
    w_d = nc.dram_tensor("w_pack", [128, 906], FP16, kind="ExternalInput")
    # col 0 = b1, col 1 rows 0:10 = b2
    b_d = nc.dram_tensor("b_pack", [128, 2], F32, kind="ExternalInput")
    z_d = nc.dram_tensor("z_t", [M2, B_LOCAL], FP16, kind="ExternalOutput")

    with tile.TileContext(nc) as tc:
        with (
            tc.tile_pool(name="static", bufs=1) as sp,
            tc.tile_pool(name="xp", bufs=NBT) as xp,
            tc.tile_pool(name="hp", bufs=4) as hp,
            # z accumulates in two one-shot tiles (no recycling: their DMAs
            # sit behind the whole x stream in the shared-bus FIFO)
            tc.tile_pool(name="zp", bufs=1) as zp,
            tc.tile_pool(name="pp1", bufs=3, space="PSUM") as pp1,
            tc.tile_pool(name="pp2", bufs=2, space="PSUM") as pp2,
            tc.tile_pool(name="ppd", bufs=1, space="PSUM") as ppd,
        ):
            # warmup operand needs no DMA: memset lets the PE start matmuls
            # almost immediately, so its 3us p-state ramp completes before
            # any real matmul's cost is locked in
            wu = sp.tile([128, NS], FP16, tag="wu")
            nc.vector.memset(wu[:], 1.0)

            # small static loads ride the SP HWDGE queue so their descriptor
            # generation overlaps the x-stream generation on GPSIMD SWDGE
            w = sp.tile([128, 906], FP16, tag="w")
            nc.sync.dma_start(w[:], w_d[:])
            b = sp.tile([128, 2], F32, tag="b")
            nc.sync.dma_start(b[:], b_d[:])
            xtail = sp.tile([KT, B_LOCAL], FP16, tag="xtail")
            nc.sync.dma_start(xtail[:], xt_d[:])

            b1 = b[:, 0:1]
            b2 = b[0:M2, 1:2]
            w1cs = [w[:, c * 128:(c + 1) * 128] for c in range(NKC)]
            wtl = w[0:KT, 768:896]
            w2 = w[:, 896:906]

            # x stream: all batch tiles issued up front (they all fit in
            # SBUF); bus serialization + tile pool deps do the pacing.
            xts = []
            for i in range(NBT):
                xt_t = xp.tile([128, NKC, BT], FP16, tag="x")
                if i == NBT - 1 and LAST_SPLIT:
                    bs = BT - LAST_SPLIT
                    nc.gpsimd.dma_start(xt_t[:, :, 0:bs], x_d[i, :, :, 0:bs])
                    nc.gpsimd.dma_start(xt_t[:, :, bs:BT], x_d[i, :, :, bs:BT])
                else:
                    nc.gpsimd.dma_start(xt_t[:], x_d[i])
                xts.append(xt_t)

            # PE warmup: keep the tensor engine busy (and its p-state
            # ramped) on junk matmuls while bt0 streams in.
            psd = ppd.tile([M1, NS], F32, tag="psd")
            for _ in range(N_WARM):
                nc.tensor.matmul(psd[:], wu[:, 0:128], wu[:],
                                 start=True, stop=True, skip_group_check=True)

            # fc2 of chunk i is deferred until after chunk i+1's fc1 so the
            # PE never waits on ACT's h output. z accumulates in two SBUF
            # tiles: bts 0-6 go out in one DMA right behind the last x
            # transfer (so no z slice ever delays the x stream on the shared
            # bus), and bt7's goes out the moment its last chunk is ready.
            zA = zp.tile([M2, (NBT - 1) * BT], FP16, tag="zA")
            zB = zp.tile([M2, BT], FP16, tag="zB")
            pending = []

            def flush_pending():
                for h_t, zt_t, lo, n, dma in pending:
                    ps2 = pp2.tile([M2, NS], F32, tag="ps2")
                    nc.tensor.matmul(ps2[:, 0:n], w2, h_t,
                                     start=True, stop=True)
                    nc.vector.tensor_scalar_add(
                        zt_t[:, lo:lo + n], ps2[:, 0:n], b2)
                    if dma is not None:
                        nc.sync.dma_start(*dma)
                pending.clear()

            for i in range(NBT):
                last = i == NBT - 1
                # chunk schedule within the bt; finish with a small chunk on
                # the last bt so the post-last-DMA drain is short
                if last and LAST_SPLIT:
                    csched = [NS] * ((BT - LAST_SPLIT) // NS) + [LAST_SPLIT]
                else:
                    csched = [NS] * (BT // NS)
                off = 0
                for ci, n in enumerate(csched):
                    goff = i * BT + off
                    ps1 = pp1.tile([M1, NS], F32, tag="ps1")
                    for c in range(NKC):
                        nc.tensor.matmul(
                            ps1[:, 0:n], w1cs[c],
                            xts[i][:, c, off:off + n],
                            start=(c == 0), stop=False)
                    nc.tensor.matmul(ps1[:, 0:n], wtl,
                                     xtail[:, goff:goff + n],
                                     start=False, stop=True)
                    h = hp.tile([M1, NS], FP16, tag="h")
                    nc.scalar.activation(
                        h[:, 0:n], ps1[:, 0:n],
                        mybir.ActivationFunctionType.Relu, bias=b1)
                    flush_pending()
                    final_chunk = ci == len(csched) - 1
                    if last:
                        dma = ((z_d[:, i * BT:], zB[:])
                               if final_chunk else None)
                        pending.append((h[:, 0:n], zB, off, n, dma))
                    else:
                        dma = ((z_d[:, 0:(NBT - 1) * BT], zA[:])
                               if i == NBT - 2 and final_chunk else None)
                        pending.append((h[:, 0:n], zA, i * BT + off, n, dma))
                    off += n
                for _ in range(PAD_SCHED[i]):
                    nc.tensor.matmul(psd[:], w[:, 0:128],
                                     xts[i][:, 0, 0:NS],
                                     start=True, stop=True,
                                     skip_group_check=True)
            flush_pending()
    nc.compile()
    return nc


def _fold_weights(conv_w, fc1_w):
    """Fold 3x3 valid cross-correlation + fc1 into one [128, 784] matrix."""
    cw = np.asarray(conv_w, np.float64)
    f1 = np.asarray(fc1_w, np.float64).reshape(M1, 26, 26)
    W = np.zeros((M1, 28, 28), np.float64)
    for di in range(3):
        for dj in range(3):
            W[:, di:di + 26, dj:dj + 26] += cw[di, dj] * f1
    return W.reshape(M1, K).astype(np.float32)


def kernel(x, conv_w, fc1_w, fc1_b, fc2_w, fc2_b):
    if "nc" not in _cache:
        _cache["nc"] = _build_nc()
    nc = _cache["nc"]

    w1t = _fold_weights(conv_w, fc1_w).T.astype(np.float16)  # [784, 128]
    w_pack = np.zeros((128, 906), np.float16)
    for c in range(NKC):
        w_pack[:, c * 128:(c + 1) * 128] = w1t[c * 128:(c + 1) * 128, :]
    w_pack[0:KT, 768:896] = w1t[KM:K, :]
    w_pack[:, 896:906] = np.asarray(fc2_w, np.float32).T.astype(np.float16)
    b_pack = np.zeros((128, 2), np.float32)
    b_pack[:, 0] = np.asarray(fc1_b, np.float32)
    b_pack[0:M2, 1] = np.asarray(fc2_b, np.float32)

    x = np.asarray(x, np.float32)
    in_maps = []
    for cid in range(N_CORES):
        xs = x[cid * B_LOCAL:(cid + 1) * B_LOCAL].T.astype(np.float16)
        # [768, 8192] -> [NBT, 128, 6, BT]: [i,p,c,j] = xs[c*128+p, i*BT+j]
        xm = xs[:KM].reshape(NKC, 128, NBT, BT).transpose(2, 1, 0, 3)
        in_maps.append({
            "x_p": np.ascontiguousarray(xm),
            "x_tail": np.ascontiguousarray(xs[KM:K]),
            "w_pack": w_pack, "b_pack": b_pack,
        })
    res = run_bass_kernel_spmd(nc, in_maps, list(range(N_CORES)))
    outs = [res.results[c]["z_t"].T for c in range(N_CORES)]
    return np.ascontiguousarray(
        np.concatenate(outs, axis=0).astype(np.float32))
